# revision 26
# baseline (speedup 1.0000x reference)
"""Differentiable Canny edge detector on 8 Trainium2 NeuronCores.

Sharding: pure data parallel over batch (32 images -> 4 per core).

Per-core pipeline (all fp32, per image):
  gray  = mean(channels)                      (DVE/gpsimd adds, DMA-loaded tiles)
  gx    = vert5_sym  . horiz5_anti (gray)     (horiz 5-tap on DVE, vert 5-tap as
  gy    = vert5_anti . horiz5_sym  (gray)      banded 128x128 matmul on PE)
  msq   = gx^2+gy^2 ; NMS compares run on msq (monotone equiv of |grad|)
  direction class from gx^2,gy^2,sign(gx*gy); neighbor max selected with
  copy_predicated cascade; row+-1 shifts via DVE stream_shuffle + tiny fix DMAs.
  mag   = sqrt(msq+1e-6); nm = keep*mag; out = nm*sigmoid(10*nm-{3,1}).

Row tiling: 5 chunks of 124 output rows (last 16), each chunk stored on 128
partitions = rows 124t-2 .. 124t+125 (2-row vertical halo baked into the DMA
loads), so the single vertical conv stage needs no cross-tile fixups.
"""

import math

import numpy as np

import concourse.bass as bass
import concourse.mybir as mybir
from concourse import bacc
from concourse.tile import TileContext

FP = mybir.dt.float32
OP = mybir.AluOpType
AF = mybir.ActivationFunctionType

P = 128
W = 512
H = 512
NT = 5                    # row chunks per image
TR = 124                  # output rows per chunk (last chunk: 16)
GW = W + 4                # gpad chunk width (2-col zero pad each side)
MW = W + 2                # msq-type chunk width (1-col pad each side, -1.0)
B_PER_CORE = 4
N_CORES = 8

# rows_out[t], and the input row span of chunk t is 124t-2 .. 124t+125
ROWS_OUT = [124, 124, 124, 124, 16]


def _banded(n, taps):
    # correlation matrix: out[y] = sum_o taps[o+k] * in[y+o], zero pad
    k = len(taps) // 2
    m = np.zeros((n, n), np.float64)
    for o in range(-k, k + 1):
        for y in range(n):
            if 0 <= y + o < n:
                m[y, y + o] = taps[o + k]
    return m


def _consts():
    e = math.exp(-0.5)
    s = 1.0 + 2.0 * e
    a = e / s          # gauss edge tap
    b = 1.0 / s        # gauss center tap
    ag = _banded(H, [a, b, a])
    # exact composition of vertical gauss then vertical sobel taps, with the
    # reference's per-stage zero padding (border rows differ from the
    # translation-invariant 5-tap)
    wx_full = (_banded(H, [1.0, 2.0, 1.0]) @ ag) * (a / 3.0)
    wy_full = (_banded(H, [-1.0, 0.0, 1.0]) @ ag) * (a / 3.0)

    def tile_w(full, t):
        w = np.zeros((P, P), np.float64)
        for m_ in range(ROWS_OUT[t]):
            row_out = TR * t + m_
            for k_ in range(P):
                row_in = TR * t - 2 + k_
                if 0 <= row_in < H:
                    w[k_, m_] = full[row_out, row_in]
        return w.astype(np.float32)

    wgx = [tile_w(wx_full, t) for t in (0, 1, 4)]   # tiles 1..3 identical
    wgy = [tile_w(wy_full, t) for t in (0, 1, 4)]
    t1sq = math.tan(math.pi / 8.0) ** 2
    t2sq = math.tan(3.0 * math.pi / 8.0) ** 2
    return (
        wgx,
        wgy,
        np.float32(b / a),        # hgauss STT ratio (hs = (b/a)*g + (gl+gr))
        np.float32(t1sq),
        np.float32(t2sq),
    )


WGX_NP, WGY_NP, R_HG, T1SQ, T2SQ = _consts()

# stream_shuffle masks (within each 32-partition quadrant)
MASK_UP = [(i + 1) % 32 for i in range(32)]     # U[p] = in[p+1]
MASK_DN = [(i + 31) % 32 for i in range(32)]    # D[p] = in[p-1]


def build_bass():
    nc = bacc.Bacc("TRN2", target_bir_lowering=False, debug=False,
                   dynamic_dma_scratch_size=4096)

    x = nc.dram_tensor("x", [B_PER_CORE, 3, H, W], FP, kind="ExternalInput")
    wgx_d = nc.dram_tensor("wgx", [3, P, P], FP, kind="ExternalInput")
    wgy_d = nc.dram_tensor("wgy", [3, P, P], FP, kind="ExternalInput")
    yhi = nc.dram_tensor("yhi", [B_PER_CORE, 1, H, W], FP, kind="ExternalOutput")
    ylo = nc.dram_tensor("ylo", [B_PER_CORE, 1, H, W], FP, kind="ExternalOutput")

    # persistent SBUF
    wgx_s = nc.alloc_sbuf_tensor("wgx_s", [P, 3, P], FP)
    wgy_s = nc.alloc_sbuf_tensor("wgy_s", [P, 3, P], FP)
    chanA = nc.alloc_sbuf_tensor("chanA", [P, NT, W], FP)
    chanB = nc.alloc_sbuf_tensor("chanB", [P, NT, W], FP)
    gpad = nc.alloc_sbuf_tensor("gpad", [P, NT, GW], FP)
    hsp = nc.alloc_sbuf_tensor("hsp", [P, NT, MW], FP)
    sA = nc.alloc_sbuf_tensor("sA", [P, NT, W], FP)
    sB = nc.alloc_sbuf_tensor("sB", [P, NT, W + 1], FP)
    hgx = nc.alloc_sbuf_tensor("hgx", [P, NT, W], FP)   # later: w = gx*gy
    hgy = nc.alloc_sbuf_tensor("hgy", [P, NT, W], FP)   # later: s-mask
    q1 = nc.alloc_sbuf_tensor("q1", [P, NT, W], FP)     # later: sigmoid(hi)
    q2 = nc.alloc_sbuf_tensor("q2", [P, NT, W], FP)     # later: sigmoid(lo)
    msqp = nc.alloc_sbuf_tensor("msqp", [P, NT, MW], FP)
    ubuf = nc.alloc_sbuf_tensor("ubuf", [P, NT, MW], FP)  # later: hi
    dbuf = nc.alloc_sbuf_tensor("dbuf", [P, NT, MW], FP)  # later: lo
    mdmag = nc.alloc_sbuf_tensor("mdmag", [P, NT, W], FP)  # Md, then mag
    cmask = nc.alloc_sbuf_tensor("cmask", [P, NT, W], mybir.dt.uint8)
    smask = nc.alloc_sbuf_tensor("smask", [P, NT, W], mybir.dt.uint8)
    nm0 = nc.alloc_sbuf_tensor("nm0", [P, NT, W], FP)
    nm1 = nc.alloc_sbuf_tensor("nm1", [P, NT, W], FP)
    negrow = nc.alloc_sbuf_tensor("negrow", [1, MW], FP)
    b_eps = nc.alloc_sbuf_tensor("b_eps", [P, 1], FP)
    b_hi = nc.alloc_sbuf_tensor("b_hi", [P, 1], FP)
    b_lo = nc.alloc_sbuf_tensor("b_lo", [P, 1], FP)

    nms = [nm0, nm1]

    with TileContext(nc) as tc:
        with tc.tile_pool(name="ps", bufs=3, space="PSUM") as psp:
            # ---- one-time init ----
            nc.sync.dma_start(wgx_s[:, :, :], wgx_d[:, :, :].rearrange("i k m -> k i m"))
            nc.sync.dma_start(wgy_s[:, :, :], wgy_d[:, :, :].rearrange("i k m -> k i m"))
            nc.vector.memset(negrow[:, :], -1.0)
            nc.vector.memset(b_eps[:, :], 1e-6)
            nc.vector.memset(b_hi[:, :], -3.0)
            nc.vector.memset(b_lo[:, :], -1.0)
            # gpad: zero everything once (dead lanes of chunk 0/4 and the
            # 2-col pads stay zero forever; live center is rewritten per image)
            nc.vector.memset(gpad[:, :, :], 0.0)
            # msq-type pads: -1.0 sentinel (strictly below any msq >= 0)
            nc.vector.memset(msqp[:, :, 0:1], -1.0)
            nc.vector.memset(msqp[:, :, MW - 1:MW], -1.0)
            # hs pads: zero (horizontal conv zero-padding)
            nc.vector.memset(hsp[:, :, 0:1], 0.0)
            nc.vector.memset(hsp[:, :, MW - 1:MW], 0.0)
            # channel buffers: zero the never-DMA'd dead regions once so the
            # gray adds can run on full partition ranges
            for cb in (chanA, chanB):
                nc.vector.memset(cb[:, :, :], 0.0)

            def chan_load(img, ch, dst):
                # chunk 0: rows 0..125 -> partitions 2..127
                nc.sync.dma_start(dst[2:128, 0, :], x[img, ch, 0:126, :])
                # chunks 1..3: rows 124t-2 .. 124t+125 (overlapping halos)
                for t in range(1, 4):
                    r0 = 124 * t - 2
                    nc.sync.dma_start(dst[:, t, :], x[img, ch, r0:r0 + 128, :])
                # chunk 4: rows 494..511 -> partitions 0..17
                nc.sync.dma_start(dst[0:18, 4, :], x[img, ch, 494:512, :])

            def phase_a(img, nm):
                # ---------------- front: gray + horizontal 5-taps ----------
                chan_load(img, 0, chanA)
                chan_load(img, 1, chanB)
                nc.gpsimd.tensor_tensor(out=sA[:, :, :], in0=chanA[:, :, :],
                                        in1=chanB[:, :, :], op=OP.add)
                chan_load(img, 2, chanA)
                nc.gpsimd.tensor_tensor(out=gpad[:, :, 2:514], in0=sA[:, :, :],
                                        in1=chanA[:, :, :], op=OP.add)

                # horizontal gauss: hs = (b/a)*g + (g[-1]+g[+1]), x(a/3) folded
                # into the PE weights
                nc.vector.tensor_tensor(out=sA[:, :, :], in0=gpad[:, :, 3:515],
                                        in1=gpad[:, :, 1:513], op=OP.add)
                nc.vector.scalar_tensor_tensor(
                    out=hsp[:, :, 1:513], in0=gpad[:, :, 2:514],
                    scalar=float(R_HG), in1=sA[:, :, :],
                    op0=OP.mult, op1=OP.add)
                # horizontal sobel parts: hgx = hs[+1]-hs[-1],
                # hgy = hs[-1]+2hs[0]+hs[+1] via two [1,1] passes
                nc.vector.tensor_tensor(out=hgx[:, :, :], in0=hsp[:, :, 2:514],
                                        in1=hsp[:, :, 0:512], op=OP.subtract)
                nc.vector.tensor_tensor(out=sB[:, :, 0:513],
                                        in0=hsp[:, :, 0:513],
                                        in1=hsp[:, :, 1:514], op=OP.add)
                nc.vector.tensor_tensor(out=hgy[:, :, :], in0=sB[:, :, 0:512],
                                        in1=sB[:, :, 1:513], op=OP.add)

                # ---------------- vertical 5-taps on PE + evictions --------
                for t in range(NT):
                    wi = {0: 0, 4: 2}.get(t, 1)
                    gxp = psp.tile([P, W], FP, tag="gx")
                    gyp = psp.tile([P, W], FP, tag="gy")
                    nc.tensor.matmul(gxp[:, :], wgx_s[:, wi, :], hgx[:, t, :],
                                     start=True, stop=True)
                    nc.tensor.matmul(gyp[:, :], wgy_s[:, wi, :], hgy[:, t, :],
                                     start=True, stop=True)
                    nc.scalar.activation(q1[:, t, :], gxp[:, :], AF.Square)
                    nc.scalar.activation(q2[:, t, :], gyp[:, :], AF.Square)
                    # w = gx*gy (only its sign is used); DVE reads at most one
                    # PSUM operand, so stage gy through SBUF
                    nc.scalar.copy(sB[:, t, 0:512], gyp[:, :])
                    nc.vector.tensor_tensor(out=hgx[:, t, :], in0=gxp[:, :],
                                            in1=sB[:, t, 0:512], op=OP.mult)

                # ---------------- NMS on squared magnitude -----------------
                nc.vector.tensor_tensor(out=msqp[:, :, 1:513], in0=q1[:, :, :],
                                        in1=q2[:, :, :], op=OP.add)
                # s-mask: 1 where gx*gy >= 0 (diag direction d1)
                nc.vector.tensor_single_scalar(
                    out=smask[:, :, :], in_=hgx[:, :, :], scalar=0.0, op=OP.is_ge)

                # row shifts: U[p]=msq[row+1], D[p]=msq[row-1]
                for t in range(NT):
                    nc.vector.stream_shuffle(ubuf[:, t, :], msqp[:, t, :], MASK_UP)
                    nc.vector.stream_shuffle(dbuf[:, t, :], msqp[:, t, :], MASK_DN)
                for r in (31, 63, 95):
                    nc.sync.dma_start(ubuf[r:r + 1, :, :], msqp[r + 1:r + 2, :, :])
                nc.sync.dma_start(ubuf[123:124, 0:4, :], msqp[0:1, 1:5, :])
                nc.sync.dma_start(ubuf[15:16, 4, :], negrow[0:1, :])
                for r in (32, 64, 96):
                    nc.sync.dma_start(dbuf[r:r + 1, :, :], msqp[r - 1:r, :, :])
                nc.sync.dma_start(dbuf[0:1, 1:5, :], msqp[123:124, 0:4, :])
                nc.sync.dma_start(dbuf[0:1, 0, :], negrow[0:1, :])

                # neighbor maxes; Md initialized with the d3 diagonal pair
                nc.vector.tensor_tensor(out=mdmag[:, :, :], in0=ubuf[:, :, 0:512],
                                        in1=dbuf[:, :, 2:514], op=OP.max)  # M3
                nc.vector.tensor_tensor(out=sA[:, :, :], in0=ubuf[:, :, 2:514],
                                        in1=dbuf[:, :, 0:512], op=OP.max)  # M1
                nc.vector.copy_predicated(out=mdmag[:, :, :], mask=smask[:, :, :],
                                          data=sA[:, :, :])
                nc.vector.tensor_tensor(out=hgx[:, :, :], in0=ubuf[:, :, 1:513],
                                        in1=dbuf[:, :, 1:513], op=OP.max)  # M2
                nc.vector.scalar_tensor_tensor(
                    out=cmask[:, :, :], in0=q1[:, :, :], scalar=float(T2SQ),
                    in1=q2[:, :, :], op0=OP.mult, op1=OP.is_lt)            # c2
                nc.vector.copy_predicated(out=mdmag[:, :, :], mask=cmask[:, :, :],
                                          data=hgx[:, :, :])
                nc.vector.tensor_tensor(out=sA[:, :, :], in0=msqp[:, :, 2:514],
                                        in1=msqp[:, :, 0:512], op=OP.max)  # M0
                nc.vector.scalar_tensor_tensor(
                    out=cmask[:, :, :], in0=q1[:, :, :], scalar=float(T1SQ),
                    in1=q2[:, :, :], op0=OP.mult, op1=OP.is_gt)            # c0
                nc.vector.copy_predicated(out=mdmag[:, :, :], mask=cmask[:, :, :],
                                          data=sA[:, :, :])
                # keep = msq > Md
                nc.vector.tensor_tensor(out=cmask[:, :, :], in0=msqp[:, :, 1:513],
                                        in1=mdmag[:, :, :], op=OP.is_gt)
                # mag = sqrt(msq + 1e-6)  (overwrites Md)
                nc.scalar.activation(mdmag[:, :, :], msqp[:, :, 1:513],
                                     AF.Sqrt, bias=b_eps[:, :])
                nc.vector.tensor_tensor(out=nm[:, :, :], in0=cmask[:, :, :],
                                        in1=mdmag[:, :, :], op=OP.mult)

            def phase_b(img, nm):
                nc.scalar.activation(q1[:, :, :], nm[:, :, :], AF.Sigmoid,
                                     bias=b_hi[:, :], scale=10.0)
                nc.scalar.activation(q2[:, :, :], nm[:, :, :], AF.Sigmoid,
                                     bias=b_lo[:, :], scale=10.0)
                nc.gpsimd.tensor_tensor(out=ubuf[:, :, 0:512], in0=nm[:, :, :],
                                        in1=q1[:, :, :], op=OP.mult)
                nc.gpsimd.tensor_tensor(out=dbuf[:, :, 0:512], in0=nm[:, :, :],
                                        in1=q2[:, :, :], op=OP.mult)
                # store: chunks 0..3 then chunk 4
                nc.sync.dma_start(
                    yhi[img, 0, 0:496, :].rearrange("(t p) w -> p t w", p=TR),
                    ubuf[0:124, 0:4, 0:512])
                nc.sync.dma_start(yhi[img, 0, 496:512, :], ubuf[0:16, 4, 0:512])
                nc.sync.dma_start(
                    ylo[img, 0, 0:496, :].rearrange("(t p) w -> p t w", p=TR),
                    dbuf[0:124, 0:4, 0:512])
                nc.sync.dma_start(ylo[img, 0, 496:512, :], dbuf[0:16, 4, 0:512])

            # pairs of images share one sqrt->sigmoid table transition
            for pair in range(B_PER_CORE // 2):
                phase_a(2 * pair, nms[0])
                phase_a(2 * pair + 1, nms[1])
                phase_b(2 * pair, nms[0])
                phase_b(2 * pair + 1, nms[1])

    nc.compile()
    return nc


_NC_CACHE = None


def _get_nc():
    global _NC_CACHE
    if _NC_CACHE is None:
        _NC_CACHE = build_bass()
    return _NC_CACHE


def kernel(x: np.ndarray):
    from concourse import bass_utils

    x = np.ascontiguousarray(np.asarray(x, dtype=np.float32))
    assert x.shape == (32, 3, H, W), x.shape
    nc = _get_nc()
    in_maps = []
    for c in range(N_CORES):
        in_maps.append({
            "x": x[c * B_PER_CORE:(c + 1) * B_PER_CORE],
            "wgx": np.stack(WGX_NP),
            "wgy": np.stack(WGY_NP),
        })
    res = bass_utils.run_bass_kernel_spmd(nc, in_maps,
                                          core_ids=list(range(N_CORES)))
    hi = np.concatenate([r["yhi"] for r in res.results], axis=0)
    lo = np.concatenate([r["ylo"] for r in res.results], axis=0)
    return hi, lo
